# revision 29
# baseline (speedup 1.0000x reference)
"""Trainium2 8-core kernel for a dense pre-norm transformer block.

Reference: h=LN1(x); qkv=h@w_qkv; causal MHA (16 heads, Dh=64);
x+=o@w_out; h2=LN2(x); x+=gelu(h2@w1+b1)@w2+b2.

Sharding (v2 — collective-minimal):
  - heads 2c,2c+1 on core c (w_qkv column-shard); each core computes
    q/k/v for ALL tokens for its 2 heads and runs their attention.
  - residual stream token-sharded: core c owns the four strided pieces
    {q*1024 + c*128 .. +128}, q=0..3.
  - after attention, the normalized head-sharded output o is exchanged
    with ONE AllToAll per batch (each core ships [1024, 256] bf16 laid
    out as 8 per-destination shards and receives exactly its own 256
    tokens x all 1024 head-dims). Out-projection, LN2, and the ENTIRE
    MLP then run token-locally with full w_out/w1/w2 resident. No
    ReduceScatter / h2-AllGather anywhere (collectives: warmup AG +
    stats AG + 2 small A2As).
  - LN1 stats shard-local via bn_stats + tiny AllGather; LN1 folded
    into the qkv matmul (rank-1 mean correction + rstd col-scaling);
    ln gains folded into w_qkv/w1 host-side; b2 added host-side.

Compute dtype: bf16 operands, fp32 PSUM. Scores computed transposed
ST=[k_pos, q_pos]; both heads packed in the PE via tile_position and
share one [128,2,512] PSUM pair so softmax exp runs as ONE scalar
activation per k-chunk; softmax denominator via a ones-column appended
to V; causality via 4 static head-doubled [128,2,512] masks; V and h2
transposes run on the DMA XBAR, not the PE.
"""
import numpy as np

import concourse.bass as bass
import concourse.mybir as mybir
import concourse.tile as tile
from concourse import bacc
from concourse import bass_utils
from concourse.masks import make_identity

F32 = mybir.dt.float32
BF = mybir.dt.bfloat16
AF = mybir.ActivationFunctionType

NCORES = 8
B, L, D = 2, 2048, 1024
T = B * L              # 4096 tokens
TSH = T // NCORES      # 512 tokens per core (4 pieces of 128)
DH = 64                # head dim
HL = 2                 # heads per core
DLOC = HL * DH         # 128 local head features
LN_EPS = 1e-5
NT = T // 512          # 8 token tiles of 512
ND = D // 128          # 8 feature chunks
QT = L // 512          # 4 q-tiles per batch
NM = 4096 // 128       # 32 MLP hidden chunks

_CACHE = {}


def build():
    if "nc" in _CACHE:
        return _CACHE["nc"]
    nc = bacc.Bacc("TRN2", target_bir_lowering=False, debug=False,
                   num_devices=NCORES)

    xt_in = nc.dram_tensor("xt", [D, T], BF, kind="ExternalInput")
    xsh_in = nc.dram_tensor("xsh", [TSH, D], BF, kind="ExternalInput")
    wqkv_in = nc.dram_tensor("wqkv", [D, 3 * DLOC], BF, kind="ExternalInput")
    nws_in = nc.dram_tensor("nws", [3 * DLOC, 1], F32, kind="ExternalInput")
    bqkv_in = nc.dram_tensor("bqkv", [3 * DLOC, 1], F32, kind="ExternalInput")
    wout_in = nc.dram_tensor("wout", [D, D], BF, kind="ExternalInput")
    w1_in = nc.dram_tensor("w1", [D, 4096], BF, kind="ExternalInput")
    b1g_in = nc.dram_tensor("b1g", [128, NM], F32, kind="ExternalInput")
    w2_in = nc.dram_tensor("w2", [4096, D], BF, kind="ExternalInput")
    masks_in = nc.dram_tensor("masks", [4, 128, 512], BF, kind="ExternalInput")
    out_ext = nc.dram_tensor("out", [TSH, D], F32, kind="ExternalOutput")

    rg = [list(range(NCORES))]

    with tile.TileContext(nc) as tc:
        with (
            tc.tile_pool(name="const", bufs=1) as const,
            tc.tile_pool(name="wmain", bufs=1) as wmain,
            tc.tile_pool(name="dram", bufs=1, space="DRAM") as dram,
        ):
            # ---- DRAM scratch for collectives ----
            st_ag_in = dram.tile([8, 128], BF)       # (piece q, mean/rstd)
            st_ag_out = dram.tile([64, 128], BF, addr_space="Shared")
            oa2a_in = [dram.tile([D, 256], BF, name=f"oa2a_in{b}")
                       for b in range(B)]
            oa2a_out = [dram.tile([D, 256], BF, name=f"oa2a_out{b}")
                        for b in range(B)]

            # ---- constants ----
            ident_bf = const.tile([128, 128], BF)
            make_identity(nc, ident_bf[:])
            ones_row = const.tile([1, 128], BF)
            nc.vector.memset(ones_row[:], 1.0)
            eps128 = const.tile([128, 1], F32)
            nc.vector.memset(eps128[:], LN_EPS)
            # head-doubled masks [128, 2, 512]
            masks_sb = [const.tile([128, 2, 512], BF, name=f"mask{m}")
                        for m in range(4)]

            def load_masks():
                for m in range(4):
                    for hl in range(HL):
                        nc.gpsimd.dma_start(masks_sb[m][:, hl, :],
                                            masks_in.ap()[m])

            # ---- persistent weights (full wout/w1; w2 in late pool) ----
            wout_sb = [wmain.tile([128, D], BF, name=f"wout{d}")
                       for d in range(ND)]
            w1_sb = [wmain.tile([128, 4096], BF, name=f"w1_{d}")
                     for d in range(ND)]
            b1g_sb = wmain.tile([128, NM], F32)

            def load_mid_weights():
                for d in range(ND):
                    nc.gpsimd.dma_start(wout_sb[d][:],
                                        wout_in.ap()[d * 128:(d + 1) * 128, :])
                for d in range(ND):
                    nc.gpsimd.dma_start(w1_sb[d][:],
                                        w1_in.ap()[d * 128:(d + 1) * 128, :])
                nc.gpsimd.dma_start(b1g_sb[:], b1g_in.ap())

            # persistent activations
            resid_pool_cm = tc.tile_pool(name="resid", bufs=1)
            resid_pool = resid_pool_cm.__enter__()
            xsv = resid_pool.tile([128, 4, D], BF)    # my shard of x
            x2_sb = resid_pool.tile([128, 4, D], BF)

            # ========== stage 0: shard-local LN1 stats + tiny AG ==========
            s0_cm = tc.tile_pool(name="s0", bufs=2)
            s0 = s0_cm.__enter__()
            ps0_cm = tc.tile_pool(name="ps0", bufs=2, space="PSUM")
            ps0 = ps0_cm.__enter__()
            for q in range(4):
                nc.scalar.dma_start(
                    xsv[:, q, :], xsh_in.ap()[q * 128:(q + 1) * 128, :])
                stats = s0.tile([128, 2, 6], F32, tag="stats", name="stats")
                xv = xsv[:, q, :].rearrange("p (s f) -> p s f", s=2)
                for s in range(2):
                    nc.vector.bn_stats(stats[:, s, :], xv[:, s, :])
                mv = s0.tile([128, 2], F32, tag="mv", name="mv")
                nc.vector.bn_aggr(mv[:], stats[:])
                rstd0 = s0.tile([128, 1], F32, tag="rstd0", name="rstd0")
                nc.scalar.activation(rstd0[:], mv[:, 1:2], AF.Sqrt,
                                     bias=eps128[:])
                nc.vector.reciprocal_approx_fast(rstd0[:], rstd0[:])
                st2 = s0.tile([128, 2], BF, tag="st2", name="st2")
                nc.vector.tensor_copy(st2[:, 0:1], mv[:, 0:1])
                nc.vector.tensor_copy(st2[:, 1:2], rstd0[:])
                stp = ps0.tile([2, 128], BF, tag="stp", name="stp")
                nc.tensor.transpose(stp[:], st2[:], ident_bf[:])
                sts = s0.tile([2, 128], BF, tag="sts", name="sts")
                nc.vector.tensor_copy(sts[:], stp[:])
                nc.scalar.dma_start(st_ag_in[2 * q:2 * q + 2, :], sts[:])
            nc.gpsimd.collective_compute(
                "AllGather", mybir.AluOpType.bypass, replica_groups=rg,
                ins=[st_ag_in[:].opt()], outs=[st_ag_out[:].opt()])
            ps0_cm.__exit__(None, None, None)
            s0_cm.__exit__(None, None, None)

            # st_ag_out rows: c*8 + q*2 + {0:mean, 1:rstd}
            st_view = st_ag_out[:].rearrange("(c x) f -> c x f", x=8)

            # ---- attention temp pools (entered early so the stage-1
            # pools above them on the stack can exit right after qkv) ----
            s2_vaug_cm = tc.tile_pool(name="s2_vaug", bufs=1)
            s2_vaug = s2_vaug_cm.__enter__()
            s2_exp_cm = tc.tile_pool(name="s2_exp", bufs=2)
            s2_exp = s2_exp_cm.__enter__()
            s2_misc_cm = tc.tile_pool(name="s2_misc", bufs=2)
            s2_misc = s2_misc_cm.__enter__()
            attn_pool_cm = tc.tile_pool(name="attn", bufs=1)
            attn_pool = attn_pool_cm.__enter__()
            qkvT = []
            for m in range(3):
                t_ = attn_pool.tile([128, T], BF, name=f"qkvT{m}")
                qkvT.append(t_)
            ps_bc_cm = tc.tile_pool(name="ps_bc", bufs=1, space="PSUM")
            ps_bc = ps_bc_cm.__enter__()

            # ================= stage 1: LN1 + qkv =================
            s1_w_cm = tc.tile_pool(name="s1_w", bufs=1)
            s1_w = s1_w_cm.__enter__()
            s1_x_cm = tc.tile_pool(name="s1_x", bufs=2)
            s1_x = s1_x_cm.__enter__()
            s1_tmp_cm = tc.tile_pool(name="s1_tmp", bufs=2)
            s1_tmp = s1_tmp_cm.__enter__()
            s1_stat_cm = tc.tile_pool(name="s1_stat", bufs=5)
            s1_stat = s1_stat_cm.__enter__()
            praw_pool_cm = tc.tile_pool(name="s1_praw", bufs=12)
            praw_pool = praw_pool_cm.__enter__()
            ps_qkv_cm = tc.tile_pool(name="ps_qkv", bufs=2, space="PSUM")
            ps_qkv = ps_qkv_cm.__enter__()
            praws = {}

            # qkv weight shard in SBUF (stage-1 scope only)
            wqkv_sb = []
            for d in range(ND):
                wt = s1_w.tile([128, 3 * DLOC], BF, name=f"wqkv{d}")
                nc.gpsimd.dma_start(wt[:],
                                    wqkv_in.ap()[d * 128:(d + 1) * 128, :])
                wqkv_sb.append(wt)
            nws_sb = []
            for m in range(3):
                nt_ = s1_w.tile([128, 1], F32, name=f"nws{m}")
                nc.gpsimd.dma_start(nt_[:],
                                    nws_in.ap()[m * 128:(m + 1) * 128, :])
                nws_sb.append(nt_)
            bqkv_sb = []
            for m in range(3):
                bt = s1_w.tile([128, 1], F32, name=f"bqkv{m}")
                nc.gpsimd.dma_start(bt[:],
                                    bqkv_in.ap()[m * 128:(m + 1) * 128, :])
                bqkv_sb.append(bt)

            def do_s1_mm(tt):
                q4, h4 = tt // 2, tt % 2
                xts = s1_x.tile([128, ND, 512], BF, tag="xts")
                for d in range(ND):
                    eng = nc.sync if d % 2 == 0 else nc.scalar
                    eng.dma_start(
                        xts[:, d, :],
                        xt_in.ap()[d * 128:(d + 1) * 128,
                                   tt * 512:(tt + 1) * 512])
                st_bf = s1_stat.tile([1, 2, 4, 128], BF, tag="st_bf",
                                     name=f"st_bf{tt}")
                nc.scalar.dma_start(
                    st_bf[:, 0, :, :], st_view[4 * h4:4 * h4 + 4, 2 * q4, :])
                nc.scalar.dma_start(
                    st_bf[:, 1, :, :],
                    st_view[4 * h4:4 * h4 + 4, 2 * q4 + 1, :])
                praws[tt] = (st_bf, [])
                for m in range(3):
                    ps_q = ps_qkv.tile([128, 512], F32, tag="ps_q",
                                       name="ps_q")
                    for d in range(ND):
                        nc.tensor.matmul(
                            ps_q[:], wqkv_sb[d][:, m * 128:(m + 1) * 128],
                            xts[:, d, :], start=(d == 0), stop=(d == ND - 1))
                    praw = praw_pool.tile([128, 512], BF, tag="praw",
                                          name="praw")
                    nc.scalar.copy(praw[:], ps_q[:])
                    praws[tt][1].append(praw)

            def do_s1_fin(tt):
                st_bf, praw3 = praws[tt]
                mean_v = st_bf[:, 0, :, :].rearrange("p a f -> p (a f)")
                rstd_v = st_bf[:, 1, :, :].rearrange("p a f -> p (a f)")
                mr = s1_stat.tile([1, 512], BF, tag="mr", name="mr", bufs=2)
                nc.vector.tensor_mul(mr[:], mean_v, rstd_v)
                rstd_b = ps_bc.tile([128, 512], F32, tag="pbc",
                                    name="rstd_b")
                nc.tensor.matmul(rstd_b[:], ones_row[:], rstd_v,
                                 start=True, stop=True)
                rstd_bc = s1_tmp.tile([128, 512], BF, tag="rstd_bc")
                nc.vector.tensor_copy(rstd_bc[:], rstd_b[:])
                mr_b = ps_bc.tile([128, 512], F32, tag="pbc", name="mr_b")
                nc.tensor.matmul(mr_b[:], ones_row[:], mr[:],
                                 start=True, stop=True)
                for m in range(3):
                    u = s1_tmp.tile([128, 512], BF, tag="pre", name="u")
                    nc.vector.tensor_mul(u[:], praw3[m][:], rstd_bc[:])
                    pre = s1_tmp.tile([128, 512], BF, tag="pre2",
                                      name="pre2")
                    nc.vector.scalar_tensor_tensor(
                        out=pre[:], in0=mr_b[:], scalar=nws_sb[m][:],
                        in1=u[:], op0=mybir.AluOpType.mult,
                        op1=mybir.AluOpType.add)
                    nc.gpsimd.tensor_scalar_add(
                        qkvT[m][:, tt * 512:(tt + 1) * 512], pre[:],
                        bqkv_sb[m][:])
                del praws[tt]

            # ---------------- stage-1 front schedule ----------------
            load_masks()
            load_mid_weights()
            for tt in range(4):
                do_s1_mm(tt)
            do_s1_fin(0)
            do_s1_mm(4)
            do_s1_fin(1)
            do_s1_mm(5)
            do_s1_fin(2)
            do_s1_mm(6)
            do_s1_fin(3)
            do_s1_mm(7)
            for tt in range(4, NT):
                do_s1_fin(tt)
            for cm in (ps_qkv_cm, praw_pool_cm, s1_stat_cm, s1_tmp_cm,
                       s1_x_cm, s1_w_cm, ps_bc_cm):
                cm.__exit__(None, None, None)

            # ============ attention ============
            ps_pair_cm = tc.tile_pool(name="ps_pair", bufs=2, space="PSUM")
            ps_pair = ps_pair_cm.__enter__()
            ps_o_cm = tc.tile_pool(name="ps_o", bufs=1, space="PSUM")
            ps_o = ps_o_cm.__enter__()
            ps_vtr_cm = tc.tile_pool(name="ps_vtr", bufs=2, space="PSUM")
            ps_vtr = ps_vtr_cm.__enter__()

            vaug_cur = [None]

            def do_attn(b, js):
                tok0 = b * L
                if js[0] == 0:
                    vaug = s2_vaug.tile([128, 2, L // 128, DH + 1], BF,
                                        tag="vaug", name="vaug")
                    nc.vector.memset(vaug[:, :, :, DH:DH + 1], 1.0)
                    vaug_cur[0] = vaug
                    # V transpose on the PE, both heads at once:
                    # [128 dloc, 128 tok] -> [128 tok, 128 dloc]
                    for kc in range(L // 128):
                        vtr = ps_vtr.tile([128, 128], BF, tag="vtr",
                                          name="vtr")
                        nc.tensor.transpose(
                            vtr[:],
                            qkvT[2][:, tok0 + kc * 128:tok0 + (kc + 1) * 128],
                            ident_bf[:])
                        nc.vector.tensor_copy(
                            vaug[:, :, kc, 0:DH],
                            vtr[:].rearrange("p (h d) -> p h d", h=2))
                vaug = vaug_cur[0]
                for j in js:
                    nk = 4 * (j + 1)
                    po = [ps_o.tile([DH + 1, 512], F32, tag=f"po{hl}",
                                    name=f"po{hl}") for hl in range(HL)]
                    for kc in range(nk):
                        dm = kc - (nk - 4)
                        col0 = 128 * dm if dm > 0 else 0
                        w = 512 - col0
                        pst2 = ps_pair.tile([128, 2, 512], F32, tag="pst2",
                                            name="pst2")
                        for hl in range(HL):
                            hrow = hl * DH
                            qsl = qkvT[0][hrow:hrow + DH,
                                          tok0 + j * 512 + col0:
                                          tok0 + (j + 1) * 512]
                            ksl = qkvT[1][hrow:hrow + DH,
                                          tok0 + kc * 128:
                                          tok0 + (kc + 1) * 128]
                            nc.tensor.matmul(pst2[:, hl, :w], ksl, qsl,
                                             start=True, stop=True,
                                             tile_position=(hrow, 0))
                        est2 = s2_exp.tile([128, 2, 512], BF, tag="est2",
                                           name="est2")
                        nc.scalar.activation(est2[:, :, :w], pst2[:, :, :w],
                                             AF.Exp, scale=0.125)
                        if dm >= 0:
                            nc.vector.tensor_mul(
                                est2[:, :, :w], est2[:, :, :w],
                                masks_sb[dm][:, :, col0:])
                        for hl in range(HL):
                            nc.tensor.matmul(po[hl][:, col0:],
                                             vaug[:, hl, kc, :],
                                             est2[:, hl, :w],
                                             start=(kc == 0),
                                             stop=(kc == nk - 1))
                    par = j // 2
                    for hl in range(HL):
                        # NOTE: reciprocal_approx_fast (custom DVE op)
                        # cannot read PSUM — copy the denominator first.
                        den = s2_misc.tile([1, 512], F32, tag="den",
                                           name="den", bufs=1)
                        nc.vector.tensor_copy(den[:], po[hl][DH:DH + 1, :])
                        rec1 = s2_misc.tile([1, 512], F32, tag="rec1",
                                            name="rec1", bufs=1)
                        nc.vector.reciprocal_approx_fast(rec1[:], den[:])
                        rec1b = s2_misc.tile([1, 512], BF, tag="rec1b",
                                             name="rec1b")
                        nc.scalar.copy(rec1b[:], rec1[:])
                        rec_sb = s2_misc.tile([64, 512], BF, tag="rec_sb",
                                              name="rec_sb")
                        nc.gpsimd.partition_broadcast(rec_sb[:], rec1b[:])
                        osl = s2_misc.tile([64, 512], BF, tag="osl",
                                           name="osl")
                        nc.vector.tensor_mul(osl[:], po[hl][0:DH, :],
                                             rec_sb[:])
                        # scatter the 4 destination-core shards:
                        # dst rows (jmod4+s)*128 + hl*64 .. +64,
                        # dst cols par*128 .. +128
                        s4base = (j % 2) * 4
                        dst = oa2a_in[b][:].rearrange(
                            "(s r) t -> r s t", r=128)[
                                hl * DH:(hl + 1) * DH,
                                s4base:s4base + 4,
                                par * 128:(par + 1) * 128]
                        src = osl[:].rearrange("d (s t) -> d s t", s=4)
                        nc.gpsimd.dma_start(dst, src)

            def fire_oa2a(b):
                nc.gpsimd.collective_compute(
                    "AllToAll", mybir.AluOpType.bypass, replica_groups=rg,
                    ins=[oa2a_in[b][:].opt()], outs=[oa2a_out[b][:].opt()])

            # ---------------- attention schedule ----------------
            do_attn(0, (0, 1, 2, 3))
            fire_oa2a(0)
            do_attn(1, (0, 1, 2, 3))
            fire_oa2a(1)

            for cm in (ps_vtr_cm, ps_o_cm, ps_pair_cm, attn_pool_cm):
                cm.__exit__(None, None, None)

            # ---- stages 3+: out-proj + LN2 + token-local MLP ----
            w2p_cm = tc.tile_pool(name="w2p", bufs=1)
            w2p = w2p_cm.__enter__()
            w2_sb = [w2p.tile([128, D], BF, name=f"w2_{m}")
                     for m in range(NM)]
            for m in range(NM):
                eng = nc.sync if m % 2 == 0 else nc.gpsimd
                eng.dma_start(w2_sb[m][:], w2_in.ap()[m * 128:(m + 1) * 128, :])

            s3_cm = tc.tile_pool(name="s3", bufs=2)
            s3 = s3_cm.__enter__()
            s3h_cm = tc.tile_pool(name="s3h", bufs=1)
            s3h = s3h_cm.__enter__()
            g1_cm = tc.tile_pool(name="g1", bufs=4)
            g1p = g1_cm.__enter__()
            out_cm = tc.tile_pool(name="outp", bufs=2)
            outp = out_cm.__enter__()
            ps_pop_cm = tc.tile_pool(name="ps_pop", bufs=1, space="PSUM")
            ps_pop = ps_pop_cm.__enter__()
            ps_m1_cm = tc.tile_pool(name="ps_m1", bufs=2, space="PSUM")
            ps_m1 = ps_m1_cm.__enter__()
            ps_m2_cm = tc.tile_pool(name="ps_m2", bufs=1, space="PSUM")
            ps_m2 = ps_m2_cm.__enter__()

            h2Ts = {}

            def og_load(q, eng):
                b, par = q // 2, q % 2
                og = s3.tile([128, ND, 128], BF, tag="og", name="og")
                for dc in range(ND):
                    eng.dma_start(
                        og[:, dc, :],
                        oa2a_out[b][dc * 128:(dc + 1) * 128,
                                    par * 128:(par + 1) * 128])
                return og

            def do_oproj(q, og):
                """out-proj + residual + LN2 + h2T for my piece q."""
                for n in range(2):
                    pop = ps_pop.tile([128, 512], F32, tag=f"pop{n}",
                                      name=f"pop{n}")
                    for dc in range(ND):
                        nc.tensor.matmul(
                            pop[:], og[:, dc, :],
                            wout_sb[dc][:, n * 512:(n + 1) * 512],
                            start=(dc == 0), stop=(dc == ND - 1))
                    nc.vector.tensor_add(
                        x2_sb[:, q, n * 512:(n + 1) * 512],
                        xsv[:, q, n * 512:(n + 1) * 512], pop[:])
                stats = s3.tile([128, 2, 6], F32, tag="stats", name="stats")
                x2v = x2_sb[:, q, :].rearrange("p (s f) -> p s f", s=2)
                for s in range(2):
                    nc.vector.bn_stats(stats[:, s, :], x2v[:, s, :])
                mv = s3.tile([128, 2], F32, tag="mv", name="mv")
                nc.vector.bn_aggr(mv[:], stats[:])
                rstd2 = s3.tile([128, 1], F32, tag="rstd2", name="rstd2")
                nc.scalar.activation(rstd2[:], mv[:, 1:2], AF.Sqrt,
                                     bias=eps128[:])
                nc.vector.reciprocal_approx_fast(rstd2[:], rstd2[:])
                h2b = s3.tile([128, D], BF, tag="h2b", name="h2b")
                nc.vector.tensor_scalar(
                    out=h2b[:], in0=x2_sb[:, q, :], scalar1=mv[:, 0:1],
                    scalar2=rstd2[:], op0=mybir.AluOpType.subtract,
                    op1=mybir.AluOpType.mult)
                h = q // 2
                if h not in h2Ts:
                    h2Ts[h] = s3h.tile([128, ND, 256], BF, tag="h2T",
                                       name="h2T")
                h2T = h2Ts[h]
                poff = (q % 2) * 128
                for dc in range(ND):
                    nc.scalar.dma_start_transpose(
                        h2T[:, dc, poff:poff + 128],
                        h2b[:, dc * 128:(dc + 1) * 128])

            def do_mlp_half(h):
                """MLP for pieces 2h, 2h+1 (my 256 tokens of batch h)."""
                h2T = h2Ts[h]
                pm2 = {}
                for p in range(2):
                    for n2 in range(2):
                        pm2[(p, n2)] = ps_m2.tile(
                            [128, 512], F32, tag=f"pm2_{p}{n2}",
                            name=f"pm2_{p}{n2}")
                g1_prev = [None]

                def mlp2(m, g1):
                    for p in range(2):
                        for n2 in range(2):
                            nc.tensor.matmul(
                                pm2[(p, n2)][:], g1[:, p * 128:(p + 1) * 128],
                                w2_sb[m][:, n2 * 512:(n2 + 1) * 512],
                                start=(m == 0), stop=(m == NM - 1))

                for m in range(NM):
                    pm1 = ps_m1.tile([128, 256], F32, tag="pm1", name="pm1")
                    for d in range(ND):
                        nc.tensor.matmul(
                            pm1[:], w1_sb[d][:, m * 128:(m + 1) * 128],
                            h2T[:, d, :], start=(d == 0), stop=(d == ND - 1))
                    g1 = g1p.tile([128, 256], BF, tag="g1", name="g1")
                    nc.scalar.activation(g1[:], pm1[:], AF.Gelu,
                                         bias=b1g_sb[:, m:m + 1])
                    if m > 0:
                        mlp2(m - 1, g1_prev[0])
                    g1_prev[0] = g1
                mlp2(NM - 1, g1_prev[0])
                for p in range(2):
                    q = 2 * h + p
                    for n2 in range(2):
                        ot = outp.tile([128, 512], F32, tag="ot", name="ot")
                        nc.vector.tensor_add(
                            ot[:], x2_sb[:, q, n2 * 512:(n2 + 1) * 512],
                            pm2[(p, n2)][:])
                        nc.sync.dma_start(
                            out_ext.ap()[q * 128:(q + 1) * 128,
                                         n2 * 512:(n2 + 1) * 512], ot[:])

            og0 = og_load(0, nc.scalar)
            og1 = og_load(1, nc.scalar)
            do_oproj(0, og0)
            do_oproj(1, og1)
            og2 = og_load(2, nc.gpsimd)
            og3 = og_load(3, nc.gpsimd)
            do_mlp_half(0)
            do_oproj(2, og2)
            do_oproj(3, og3)
            do_mlp_half(1)

            for cm in (ps_m2_cm, ps_m1_cm, ps_pop_cm, out_cm, g1_cm,
                       s3h_cm, s3_cm, w2p_cm, s2_misc_cm, s2_exp_cm,
                       s2_vaug_cm, resid_pool_cm):
                cm.__exit__(None, None, None)

    nc.compile()
    _CACHE["nc"] = nc
    return nc


def shard_rows(c):
    """Global token rows owned by core c (four strided pieces of 128)."""
    return np.concatenate(
        [np.arange(q * 1024 + c * 128, q * 1024 + (c + 1) * 128)
         for q in range(4)])


def make_in_maps(x, ln1_g, ln1_b, w_qkv, w_out, ln2_g, ln2_b, w1, b1, w2, b2):
    import ml_dtypes
    bf16 = ml_dtypes.bfloat16
    x = np.asarray(x, np.float32)
    xf = np.ascontiguousarray(x.reshape(T, D))
    xt = np.ascontiguousarray(xf.T.astype(bf16))
    w_qkv_eff = np.asarray(w_qkv) * np.asarray(ln1_g)[:, None]
    bias_qkv = np.asarray(ln1_b) @ np.asarray(w_qkv)
    w1_eff = np.ascontiguousarray(
        (np.asarray(w1) * np.asarray(ln2_g)[:, None]).astype(bf16))
    bias_h1 = np.asarray(ln2_b) @ np.asarray(w1) + np.asarray(b1)
    b1g = np.ascontiguousarray(
        bias_h1.reshape(NM, 128).T.astype(np.float32))
    w_out_b = np.ascontiguousarray(np.asarray(w_out).astype(bf16))
    w2_b = np.ascontiguousarray(np.asarray(w2).astype(bf16))
    km = np.arange(128)[:, None]
    qm = np.arange(512)[None, :]
    masks = np.stack([(km + 128 * m <= qm).astype(bf16)
                      for m in range(4)])
    in_maps = []
    for c in range(NCORES):
        cs = slice(c * DLOC, (c + 1) * DLOC)
        wq = np.concatenate(
            [w_qkv_eff[:, cs], w_qkv_eff[:, D:][:, cs],
             w_qkv_eff[:, 2 * D:][:, cs]], axis=1)
        bq = np.concatenate(
            [bias_qkv[cs], bias_qkv[D:][cs], bias_qkv[2 * D:][cs]])
        rows = shard_rows(c)
        in_maps.append({
            "xt": xt,
            "xsh": np.ascontiguousarray(xf[rows].astype(bf16)),
            "wqkv": np.ascontiguousarray(wq.astype(bf16)),
            "nws": np.ascontiguousarray(
                (-wq.astype(np.float32).sum(axis=0)).astype(
                    np.float32)).reshape(-1, 1),
            "bqkv": np.ascontiguousarray(bq, np.float32).reshape(-1, 1),
            "wout": w_out_b,
            "w1": w1_eff,
            "b1g": b1g,
            "w2": w2_b,
            "masks": masks,
        })
    return in_maps


def kernel(**inputs):
    nc = build()
    in_maps = make_in_maps(**inputs)
    res = bass_utils.run_bass_kernel_spmd(
        nc, in_maps, core_ids=list(range(NCORES)))
    out = np.empty((T, D), np.float32)
    for c in range(NCORES):
        out[shard_rows(c)] = res.results[c]["out"]
    out += np.asarray(inputs["b2"], np.float32)[None, :]
    return out.reshape(B, L, D).astype(np.float32)


# revision 40
# speedup vs baseline: 1.1828x; 1.1828x over previous
"""Trainium2 8-core kernel for a dense pre-norm transformer block.

Reference: h=LN1(x); qkv=h@w_qkv; causal MHA (16 heads, Dh=64);
x+=o@w_out; h2=LN2(x); x+=gelu(h2@w1+b1)@w2+b2.

Sharding (v2 — collective-minimal):
  - heads 2c,2c+1 on core c (w_qkv column-shard); each core computes
    q/k/v for ALL tokens for its 2 heads and runs their attention.
  - residual stream token-sharded: core c owns the four strided pieces
    {q*1024 + c*128 .. +128}, q=0..3.
  - after attention, the normalized head-sharded output o is exchanged
    with ONE AllToAll per batch (each core ships [1024, 256] bf16 laid
    out as 8 per-destination shards and receives exactly its own 256
    tokens x all 1024 head-dims). Out-projection, LN2, and the ENTIRE
    MLP then run token-locally with full w_out/w1/w2 resident. No
    ReduceScatter / h2-AllGather anywhere (collectives: warmup AG +
    stats AG + 2 small A2As).
  - LN1 stats shard-local via bn_stats + tiny AllGather; LN1 folded
    into the qkv matmul (rank-1 mean correction + rstd col-scaling);
    ln gains folded into w_qkv/w1 host-side; b2 added host-side.

Compute dtype: bf16 operands, fp32 PSUM. Scores computed transposed
ST=[k_pos, q_pos]; both heads packed in the PE via tile_position and
share one [128,2,512] PSUM pair so softmax exp runs as ONE scalar
activation per k-chunk; softmax denominator via a ones-column appended
to V; causality via 4 static head-doubled [128,2,512] masks; V and h2
transposes run on the DMA XBAR, not the PE.
"""
import numpy as np

import concourse.bass as bass
import concourse.mybir as mybir
import concourse.tile as tile
from concourse import bacc
from concourse import bass_utils
from concourse.masks import make_identity

F32 = mybir.dt.float32
BF = mybir.dt.bfloat16
AF = mybir.ActivationFunctionType

NCORES = 8
B, L, D = 2, 2048, 1024
T = B * L              # 4096 tokens
TSH = T // NCORES      # 512 tokens per core (4 pieces of 128)
DH = 64                # head dim
HL = 2                 # heads per core
DLOC = HL * DH         # 128 local head features
LN_EPS = 1e-5
NT = T // 512          # 8 token tiles of 512
ND = D // 128          # 8 feature chunks
QT = L // 512          # 4 q-tiles per batch
NM = 4096 // 128       # 32 MLP hidden chunks

_CACHE = {}


def build():
    if "nc" in _CACHE:
        return _CACHE["nc"]
    nc = bacc.Bacc("TRN2", target_bir_lowering=False, debug=False,
                   num_devices=NCORES)

    xt_in = nc.dram_tensor("xt", [D, T], BF, kind="ExternalInput")
    xsh_in = nc.dram_tensor("xsh", [TSH, D], BF, kind="ExternalInput")
    wqkv_in = nc.dram_tensor("wqkv", [D, 3 * DLOC], BF, kind="ExternalInput")
    nws_in = nc.dram_tensor("nws", [3 * DLOC, 1], F32, kind="ExternalInput")
    bqkv_in = nc.dram_tensor("bqkv", [3, DLOC], BF, kind="ExternalInput")
    wout_in = nc.dram_tensor("wout", [D, D], BF, kind="ExternalInput")
    w1_in = nc.dram_tensor("w1", [D, 4096], BF, kind="ExternalInput")
    b1g_in = nc.dram_tensor("b1g", [128, NM], F32, kind="ExternalInput")
    w2_in = nc.dram_tensor("w2", [4096, D], BF, kind="ExternalInput")
    masks_in = nc.dram_tensor("masks", [4, 128, 512], BF, kind="ExternalInput")
    out_ext = nc.dram_tensor("out", [TSH, D], F32, kind="ExternalOutput")

    rg = [list(range(NCORES))]

    with tile.TileContext(nc) as tc:
        with (
            tc.tile_pool(name="const", bufs=1) as const,
            tc.tile_pool(name="wmain", bufs=1) as wmain,
            tc.tile_pool(name="dram", bufs=1, space="DRAM") as dram,
        ):
            # ---- DRAM scratch for collectives ----
            st_ag_in = dram.tile([8, 128], BF)       # (piece q, mean/rstd)
            st_ag_out = dram.tile([64, 128], BF, addr_space="Shared")
            oa2a_in = [dram.tile([D, 256], BF, name=f"oa2a_in{b}")
                       for b in range(B)]
            oa2a_out = [dram.tile([D, 256], BF, name=f"oa2a_out{b}")
                        for b in range(B)]
            warm_in = dram.tile([8, 16], BF)
            warm_out = dram.tile([64, 16], BF, addr_space="Shared")

            # ---- constants ----
            ident_bf = const.tile([128, 128], BF)
            make_identity(nc, ident_bf[:])
            ones_row = const.tile([1, 128], BF)
            nc.vector.memset(ones_row[:], 1.0)
            ones512 = const.tile([1, 512], BF)
            nc.vector.memset(ones512[:], 1.0)
            eps128 = const.tile([128, 1], F32)
            nc.vector.memset(eps128[:], LN_EPS)
            # additive causal masks (0 / -800, pre-softmax-scale)
            masks_sb = [const.tile([128, 512], BF, name=f"mask{m}")
                        for m in range(4)]

            def load_masks():
                for m in range(4):
                    nc.gpsimd.dma_start(masks_sb[m][:], masks_in.ap()[m])

            # ---- persistent weights (full wout/w1; w2 in late pool) ----
            wout_sb = [wmain.tile([128, D], BF, name=f"wout{d}")
                       for d in range(ND)]
            w1_sb = [wmain.tile([128, 4096], BF, name=f"w1_{d}")
                     for d in range(ND)]
            b1g_sb = wmain.tile([128, NM], F32)

            def load_mid_weights():
                for d in range(ND):
                    nc.gpsimd.dma_start(wout_sb[d][:],
                                        wout_in.ap()[d * 128:(d + 1) * 128, :])
                for d in range(ND):
                    nc.gpsimd.dma_start(w1_sb[d][:],
                                        w1_in.ap()[d * 128:(d + 1) * 128, :])
                nc.gpsimd.dma_start(b1g_sb[:], b1g_in.ap())

            # persistent activations
            resid_pool_cm = tc.tile_pool(name="resid", bufs=1)
            resid_pool = resid_pool_cm.__enter__()
            xsv = resid_pool.tile([128, 4, D], BF)    # my shard of x
            x2_sb = resid_pool.tile([128, 4, D], BF)

            # ========== stage 0: shard-local LN1 stats + tiny AG ==========
            s0_cm = tc.tile_pool(name="s0", bufs=2)
            s0 = s0_cm.__enter__()
            ps0_cm = tc.tile_pool(name="ps0", bufs=2, space="PSUM")
            ps0 = ps0_cm.__enter__()
            # absorb first-collective init latency with a no-dep dummy
            wtile = s0.tile([8, 16], BF, tag="wtile", name="wtile")
            nc.vector.memset(wtile[:], 0.0)
            nc.scalar.dma_start(warm_in[:], wtile[:])
            nc.gpsimd.collective_compute(
                "AllGather", mybir.AluOpType.bypass, replica_groups=rg,
                ins=[warm_in[:].opt()], outs=[warm_out[:].opt()])
            for q in range(4):
                nc.scalar.dma_start(
                    xsv[:, q, :], xsh_in.ap()[q * 128:(q + 1) * 128, :])
                stats = s0.tile([128, 2, 6], F32, tag="stats", name="stats")
                xv = xsv[:, q, :].rearrange("p (s f) -> p s f", s=2)
                for s in range(2):
                    nc.vector.bn_stats(stats[:, s, :], xv[:, s, :])
                mv = s0.tile([128, 2], F32, tag="mv", name="mv")
                nc.vector.bn_aggr(mv[:], stats[:])
                rstd0 = s0.tile([128, 1], F32, tag="rstd0", name="rstd0")
                nc.scalar.activation(rstd0[:], mv[:, 1:2], AF.Sqrt,
                                     bias=eps128[:])
                nc.vector.reciprocal_approx_fast(rstd0[:], rstd0[:])
                st2 = s0.tile([128, 2], BF, tag="st2", name="st2")
                nc.vector.tensor_copy(st2[:, 0:1], mv[:, 0:1])
                nc.vector.tensor_copy(st2[:, 1:2], rstd0[:])
                stp = ps0.tile([2, 128], BF, tag="stp", name="stp")
                nc.tensor.transpose(stp[:], st2[:], ident_bf[:])
                sts = s0.tile([2, 128], BF, tag="sts", name="sts")
                nc.vector.tensor_copy(sts[:], stp[:])
                nc.scalar.dma_start(st_ag_in[2 * q:2 * q + 2, :], sts[:])
            nc.gpsimd.collective_compute(
                "AllGather", mybir.AluOpType.bypass, replica_groups=rg,
                ins=[st_ag_in[:].opt()], outs=[st_ag_out[:].opt()])
            ps0_cm.__exit__(None, None, None)
            s0_cm.__exit__(None, None, None)

            # st_ag_out rows: c*8 + q*2 + {0:mean, 1:rstd}
            st_view = st_ag_out[:].rearrange("(c x) f -> c x f", x=8)

            # ---- attention temp pools (entered early so the stage-1
            # pools above them on the stack can exit right after qkv) ----
            s2_vaug_cm = tc.tile_pool(name="s2_vaug", bufs=1)
            s2_vaug = s2_vaug_cm.__enter__()
            s2_exp_cm = tc.tile_pool(name="s2_exp", bufs=2)
            s2_exp = s2_exp_cm.__enter__()
            s2_misc_cm = tc.tile_pool(name="s2_misc", bufs=2)
            s2_misc = s2_misc_cm.__enter__()
            attn_pool_cm = tc.tile_pool(name="attn", bufs=1)
            attn_pool = attn_pool_cm.__enter__()
            qkvT = []
            for m in range(3):
                t_ = attn_pool.tile([128, T], BF, name=f"qkvT{m}")
                qkvT.append(t_)
            ps_bc_cm = tc.tile_pool(name="ps_bc", bufs=1, space="PSUM")
            ps_bc = ps_bc_cm.__enter__()

            # ================= stage 1: LN1 + qkv =================
            s1_w_cm = tc.tile_pool(name="s1_w", bufs=1)
            s1_w = s1_w_cm.__enter__()
            s1_x_cm = tc.tile_pool(name="s1_x", bufs=2)
            s1_x = s1_x_cm.__enter__()
            s1_tmp_cm = tc.tile_pool(name="s1_tmp", bufs=2)
            s1_tmp = s1_tmp_cm.__enter__()
            s1_stat_cm = tc.tile_pool(name="s1_stat", bufs=5)
            s1_stat = s1_stat_cm.__enter__()
            praw_pool_cm = tc.tile_pool(name="s1_praw", bufs=12)
            praw_pool = praw_pool_cm.__enter__()
            ps_qkv_cm = tc.tile_pool(name="ps_qkv", bufs=2, space="PSUM")
            ps_qkv = ps_qkv_cm.__enter__()
            praws = {}

            # qkv weight shard in SBUF (stage-1 scope only)
            wqkv_sb = []
            for d in range(ND):
                wt = s1_w.tile([128, 3 * DLOC], BF, name=f"wqkv{d}")
                nc.gpsimd.dma_start(wt[:],
                                    wqkv_in.ap()[d * 128:(d + 1) * 128, :])
                wqkv_sb.append(wt)
            nws_sb = []
            for m in range(3):
                nt_ = s1_w.tile([128, 1], F32, name=f"nws{m}")
                nc.gpsimd.dma_start(nt_[:],
                                    nws_in.ap()[m * 128:(m + 1) * 128, :])
                nws_sb.append(nt_)
            bqkv_sb = []
            for m in range(3):
                bt = s1_w.tile([1, DLOC], BF, name=f"bqkv{m}")
                nc.gpsimd.dma_start(bt[:], bqkv_in.ap()[m:m + 1, :])
                bqkv_sb.append(bt)

            def do_s1_mm(tt):
                q4, h4 = tt // 2, tt % 2
                xts = s1_x.tile([128, ND, 512], BF, tag="xts")
                for d in range(ND):
                    eng = nc.sync if d % 2 == 0 else nc.scalar
                    eng.dma_start(
                        xts[:, d, :],
                        xt_in.ap()[d * 128:(d + 1) * 128,
                                   tt * 512:(tt + 1) * 512])
                st_bf = s1_stat.tile([1, 2, 4, 128], BF, tag="st_bf",
                                     name=f"st_bf{tt}")
                nc.scalar.dma_start(
                    st_bf[:, 0, :, :], st_view[4 * h4:4 * h4 + 4, 2 * q4, :])
                nc.scalar.dma_start(
                    st_bf[:, 1, :, :],
                    st_view[4 * h4:4 * h4 + 4, 2 * q4 + 1, :])
                praws[tt] = (st_bf, [])
                for m in range(3):
                    ps_q = ps_qkv.tile([128, 512], F32, tag="ps_q",
                                       name="ps_q")
                    for d in range(ND):
                        nc.tensor.matmul(
                            ps_q[:], wqkv_sb[d][:, m * 128:(m + 1) * 128],
                            xts[:, d, :], start=(d == 0), stop=False)
                    # rank-1 bias add: bqkv[m] x ones
                    nc.tensor.matmul(ps_q[:], bqkv_sb[m][:], ones512[:],
                                     start=False, stop=True)
                    praw = praw_pool.tile([128, 512], BF, tag="praw",
                                          name="praw")
                    nc.scalar.copy(praw[:], ps_q[:])
                    praws[tt][1].append(praw)

            def do_s1_fin(tt):
                st_bf, praw3 = praws[tt]
                mean_v = st_bf[:, 0, :, :].rearrange("p a f -> p (a f)")
                rstd_v = st_bf[:, 1, :, :].rearrange("p a f -> p (a f)")
                mr = s1_stat.tile([1, 512], BF, tag="mr", name="mr", bufs=2)
                nc.vector.tensor_mul(mr[:], mean_v, rstd_v)
                rstd_b = ps_bc.tile([128, 512], F32, tag="pbc",
                                    name="rstd_b")
                nc.tensor.matmul(rstd_b[:], ones_row[:], rstd_v,
                                 start=True, stop=True)
                rstd_bc = s1_tmp.tile([128, 512], BF, tag="rstd_bc")
                nc.vector.tensor_copy(rstd_bc[:], rstd_b[:])
                mr_b = ps_bc.tile([128, 512], F32, tag="pbc", name="mr_b")
                nc.tensor.matmul(mr_b[:], ones_row[:], mr[:],
                                 start=True, stop=True)
                for m in range(3):
                    u = s1_tmp.tile([128, 512], BF, tag="pre", name="u")
                    nc.vector.tensor_mul(u[:], praw3[m][:], rstd_bc[:])
                    nc.vector.scalar_tensor_tensor(
                        out=qkvT[m][:, tt * 512:(tt + 1) * 512],
                        in0=mr_b[:], scalar=nws_sb[m][:],
                        in1=u[:], op0=mybir.AluOpType.mult,
                        op1=mybir.AluOpType.add)
                del praws[tt]

            # ---------------- stage-1 front schedule ----------------
            load_masks()
            load_mid_weights()
            for tt in range(4):
                do_s1_mm(tt)
            do_s1_fin(0)
            do_s1_mm(4)
            do_s1_fin(1)
            do_s1_mm(5)
            do_s1_fin(2)
            do_s1_mm(6)
            do_s1_fin(3)
            do_s1_mm(7)
            for tt in range(4, NT):
                do_s1_fin(tt)
            for cm in (ps_qkv_cm, praw_pool_cm, s1_stat_cm, s1_tmp_cm,
                       s1_x_cm, s1_w_cm, ps_bc_cm):
                cm.__exit__(None, None, None)

            # ============ attention ============
            ps_pair_cm = tc.tile_pool(name="ps_pair", bufs=2, space="PSUM")
            ps_pair = ps_pair_cm.__enter__()
            ps_o_cm = tc.tile_pool(name="ps_o", bufs=1, space="PSUM")
            ps_o = ps_o_cm.__enter__()
            ps_vtr_cm = tc.tile_pool(name="ps_vtr", bufs=2, space="PSUM")
            ps_vtr = ps_vtr_cm.__enter__()

            vaug_cur = [None]

            def do_attn(b, js):
                tok0 = b * L
                if js[0] == 0:
                    vaug = s2_vaug.tile([128, 2, L // 128, DH + 1], BF,
                                        tag="vaug", name="vaug")
                    nc.vector.memset(vaug[:, :, :, DH:DH + 1], 1.0)
                    vaug_cur[0] = vaug
                    # V transpose on the PE, both heads at once:
                    # [128 dloc, 128 tok] -> [128 tok, 128 dloc]
                    for kc in range(L // 128):
                        vtr = ps_vtr.tile([128, 128], BF, tag="vtr",
                                          name="vtr")
                        nc.tensor.transpose(
                            vtr[:],
                            qkvT[2][:, tok0 + kc * 128:tok0 + (kc + 1) * 128],
                            ident_bf[:])
                        nc.vector.tensor_copy(
                            vaug[:, :, kc, 0:DH],
                            vtr[:].rearrange("p (h d) -> p h d", h=2))
                vaug = vaug_cur[0]
                for j in js:
                    nk = 4 * (j + 1)
                    po = [ps_o.tile([DH + 1, 512], F32, tag=f"po{hl}",
                                    name=f"po{hl}") for hl in range(HL)]
                    for kc in range(nk):
                        dm = kc - (nk - 4)
                        col0 = 128 * dm if dm > 0 else 0
                        w = 512 - col0
                        pst2 = ps_pair.tile([128, 2, 512], F32, tag="pst2",
                                            name="pst2")
                        for hl in range(HL):
                            hrow = hl * DH
                            qsl = qkvT[0][hrow:hrow + DH,
                                          tok0 + j * 512 + col0:
                                          tok0 + (j + 1) * 512]
                            ksl = qkvT[1][hrow:hrow + DH,
                                          tok0 + kc * 128:
                                          tok0 + (kc + 1) * 128]
                            nc.tensor.matmul(pst2[:, hl, :w], ksl, qsl,
                                             start=True, stop=(dm < 0),
                                             tile_position=(hrow, 0))
                            if dm >= 0:
                                # additive -800 causal mask via the PE
                                nc.tensor.matmul(
                                    pst2[:, hl, :w], ident_bf[:],
                                    masks_sb[dm][:, col0:],
                                    start=False, stop=True)
                        est2 = s2_exp.tile([128, 2, 512], BF, tag="est2",
                                           name="est2")
                        nc.scalar.activation(est2[:, :, :w], pst2[:, :, :w],
                                             AF.Exp, scale=0.125)
                        for hl in range(HL):
                            nc.tensor.matmul(po[hl][:, col0:],
                                             vaug[:, hl, kc, :],
                                             est2[:, hl, :w],
                                             start=(kc == 0),
                                             stop=(kc == nk - 1))
                    par = j // 2
                    for hl in range(HL):
                        # NOTE: reciprocal_approx_fast (custom DVE op)
                        # cannot read PSUM — copy the denominator first.
                        den = s2_misc.tile([1, 512], F32, tag="den",
                                           name="den", bufs=1)
                        nc.vector.tensor_copy(den[:], po[hl][DH:DH + 1, :])
                        rec1 = s2_misc.tile([1, 512], F32, tag="rec1",
                                            name="rec1", bufs=1)
                        nc.vector.reciprocal_approx_fast(rec1[:], den[:])
                        rec1b = s2_misc.tile([1, 512], BF, tag="rec1b",
                                             name="rec1b")
                        nc.scalar.copy(rec1b[:], rec1[:])
                        rec_sb = s2_misc.tile([64, 512], BF, tag="rec_sb",
                                              name="rec_sb")
                        nc.gpsimd.partition_broadcast(rec_sb[:], rec1b[:])
                        osl = s2_misc.tile([64, 512], BF, tag="osl",
                                           name="osl")
                        nc.vector.tensor_mul(osl[:], po[hl][0:DH, :],
                                             rec_sb[:])
                        # scatter the 4 destination-core shards:
                        # dst rows (jmod4+s)*128 + hl*64 .. +64,
                        # dst cols par*128 .. +128
                        s4base = (j % 2) * 4
                        dst = oa2a_in[b][:].rearrange(
                            "(s r) t -> r s t", r=128)[
                                hl * DH:(hl + 1) * DH,
                                s4base:s4base + 4,
                                par * 128:(par + 1) * 128]
                        src = osl[:].rearrange("d (s t) -> d s t", s=4)
                        nc.gpsimd.dma_start(dst, src)

            def fire_oa2a(b):
                nc.gpsimd.collective_compute(
                    "AllToAll", mybir.AluOpType.bypass, replica_groups=rg,
                    ins=[oa2a_in[b][:].opt()], outs=[oa2a_out[b][:].opt()])

            # ---------------- attention schedule ----------------
            do_attn(0, (0, 1, 2, 3))
            fire_oa2a(0)
            do_attn(1, (0, 1, 2, 3))
            fire_oa2a(1)

            for cm in (ps_vtr_cm, ps_o_cm, ps_pair_cm, attn_pool_cm):
                cm.__exit__(None, None, None)

            # ---- stages 3+: out-proj + LN2 + token-local MLP ----
            w2p_cm = tc.tile_pool(name="w2p", bufs=1)
            w2p = w2p_cm.__enter__()
            w2_sb = [w2p.tile([128, D], BF, name=f"w2_{m}")
                     for m in range(NM)]
            for m in range(NM):
                eng = nc.sync if m % 2 == 0 else nc.gpsimd
                eng.dma_start(w2_sb[m][:], w2_in.ap()[m * 128:(m + 1) * 128, :])

            s3_cm = tc.tile_pool(name="s3", bufs=2)
            s3 = s3_cm.__enter__()
            s3h_cm = tc.tile_pool(name="s3h", bufs=1)
            s3h = s3h_cm.__enter__()
            g1_cm = tc.tile_pool(name="g1", bufs=4)
            g1p = g1_cm.__enter__()
            out_cm = tc.tile_pool(name="outp", bufs=2)
            outp = out_cm.__enter__()
            ps_pop_cm = tc.tile_pool(name="ps_pop", bufs=1, space="PSUM")
            ps_pop = ps_pop_cm.__enter__()
            ps_m1_cm = tc.tile_pool(name="ps_m1", bufs=2, space="PSUM")
            ps_m1 = ps_m1_cm.__enter__()
            ps_m2_cm = tc.tile_pool(name="ps_m2", bufs=1, space="PSUM")
            ps_m2 = ps_m2_cm.__enter__()

            h2Ts = {}

            def og_load(q, eng):
                b, par = q // 2, q % 2
                og = s3.tile([128, ND, 128], BF, tag="og", name="og")
                for dc in range(ND):
                    eng.dma_start(
                        og[:, dc, :],
                        oa2a_out[b][dc * 128:(dc + 1) * 128,
                                    par * 128:(par + 1) * 128])
                return og

            def do_oproj(q, og):
                """out-proj + residual + LN2 + h2T for my piece q."""
                for n in range(2):
                    pop = ps_pop.tile([128, 512], F32, tag=f"pop{n}",
                                      name=f"pop{n}")
                    for dc in range(ND):
                        nc.tensor.matmul(
                            pop[:], og[:, dc, :],
                            wout_sb[dc][:, n * 512:(n + 1) * 512],
                            start=(dc == 0), stop=(dc == ND - 1))
                    nc.vector.tensor_add(
                        x2_sb[:, q, n * 512:(n + 1) * 512],
                        xsv[:, q, n * 512:(n + 1) * 512], pop[:])
                stats = s3.tile([128, 2, 6], F32, tag="stats", name="stats")
                x2v = x2_sb[:, q, :].rearrange("p (s f) -> p s f", s=2)
                for s in range(2):
                    nc.vector.bn_stats(stats[:, s, :], x2v[:, s, :])
                mv = s3.tile([128, 2], F32, tag="mv", name="mv")
                nc.vector.bn_aggr(mv[:], stats[:])
                rstd2 = s3.tile([128, 1], F32, tag="rstd2", name="rstd2")
                nc.scalar.activation(rstd2[:], mv[:, 1:2], AF.Sqrt,
                                     bias=eps128[:])
                nc.vector.reciprocal_approx_fast(rstd2[:], rstd2[:])
                h2b = s3.tile([128, D], BF, tag="h2b", name="h2b")
                nc.vector.tensor_scalar(
                    out=h2b[:], in0=x2_sb[:, q, :], scalar1=mv[:, 0:1],
                    scalar2=rstd2[:], op0=mybir.AluOpType.subtract,
                    op1=mybir.AluOpType.mult)
                h = q // 2
                if h not in h2Ts:
                    h2Ts[h] = s3h.tile([128, ND, 256], BF, tag="h2T",
                                       name="h2T")
                h2T = h2Ts[h]
                poff = (q % 2) * 128
                for dc in range(ND):
                    nc.sync.dma_start_transpose(
                        h2T[:, dc, poff:poff + 128],
                        h2b[:, dc * 128:(dc + 1) * 128])

            def do_mlp_half(h):
                """MLP for pieces 2h, 2h+1 (my 256 tokens of batch h)."""
                h2T = h2Ts[h]
                pm2 = {}
                for p in range(2):
                    for n2 in range(2):
                        pm2[(p, n2)] = ps_m2.tile(
                            [128, 512], F32, tag=f"pm2_{p}{n2}",
                            name=f"pm2_{p}{n2}")
                g1_prev = [None]

                def mlp2(m, g1):
                    for p in range(2):
                        for n2 in range(2):
                            nc.tensor.matmul(
                                pm2[(p, n2)][:], g1[:, p * 128:(p + 1) * 128],
                                w2_sb[m][:, n2 * 512:(n2 + 1) * 512],
                                start=(m == 0), stop=(m == NM - 1))

                for m in range(NM):
                    pm1 = ps_m1.tile([128, 256], F32, tag="pm1", name="pm1")
                    for d in range(ND):
                        nc.tensor.matmul(
                            pm1[:], w1_sb[d][:, m * 128:(m + 1) * 128],
                            h2T[:, d, :], start=(d == 0), stop=(d == ND - 1))
                    g1 = g1p.tile([128, 256], BF, tag="g1", name="g1")
                    nc.scalar.activation(g1[:], pm1[:], AF.Gelu,
                                         bias=b1g_sb[:, m:m + 1])
                    if m > 0:
                        mlp2(m - 1, g1_prev[0])
                    g1_prev[0] = g1
                mlp2(NM - 1, g1_prev[0])
                for p in range(2):
                    q = 2 * h + p
                    for n2 in range(2):
                        ot = outp.tile([128, 512], F32, tag="ot", name="ot")
                        nc.vector.tensor_add(
                            ot[:], x2_sb[:, q, n2 * 512:(n2 + 1) * 512],
                            pm2[(p, n2)][:])
                        nc.sync.dma_start(
                            out_ext.ap()[q * 128:(q + 1) * 128,
                                         n2 * 512:(n2 + 1) * 512], ot[:])

            og0 = og_load(0, nc.scalar)
            og1 = og_load(1, nc.scalar)
            do_oproj(0, og0)
            do_oproj(1, og1)
            og2 = og_load(2, nc.gpsimd)
            og3 = og_load(3, nc.gpsimd)
            do_mlp_half(0)
            do_oproj(2, og2)
            do_oproj(3, og3)
            do_mlp_half(1)

            for cm in (ps_m2_cm, ps_m1_cm, ps_pop_cm, out_cm, g1_cm,
                       s3h_cm, s3_cm, w2p_cm, s2_misc_cm, s2_exp_cm,
                       s2_vaug_cm, resid_pool_cm):
                cm.__exit__(None, None, None)

    nc.compile()
    _CACHE["nc"] = nc
    return nc


def shard_rows(c):
    """Global token rows owned by core c (four strided pieces of 128)."""
    return np.concatenate(
        [np.arange(q * 1024 + c * 128, q * 1024 + (c + 1) * 128)
         for q in range(4)])


def make_in_maps(x, ln1_g, ln1_b, w_qkv, w_out, ln2_g, ln2_b, w1, b1, w2, b2):
    import ml_dtypes
    bf16 = ml_dtypes.bfloat16
    x = np.asarray(x, np.float32)
    xf = np.ascontiguousarray(x.reshape(T, D))
    xt = np.ascontiguousarray(xf.T.astype(bf16))
    w_qkv_eff = np.asarray(w_qkv) * np.asarray(ln1_g)[:, None]
    bias_qkv = np.asarray(ln1_b) @ np.asarray(w_qkv)
    w1_eff = np.ascontiguousarray(
        (np.asarray(w1) * np.asarray(ln2_g)[:, None]).astype(bf16))
    bias_h1 = np.asarray(ln2_b) @ np.asarray(w1) + np.asarray(b1)
    b1g = np.ascontiguousarray(
        bias_h1.reshape(NM, 128).T.astype(np.float32))
    w_out_b = np.ascontiguousarray(np.asarray(w_out).astype(bf16))
    w2_b = np.ascontiguousarray(np.asarray(w2).astype(bf16))
    km = np.arange(128)[:, None]
    qm = np.arange(512)[None, :]
    # additive masks: 0 where attended, -800 where causally masked
    masks = np.stack([
        np.where(km + 128 * m <= qm, 0.0, -800.0).astype(bf16)
        for m in range(4)])
    in_maps = []
    for c in range(NCORES):
        cs = slice(c * DLOC, (c + 1) * DLOC)
        wq = np.concatenate(
            [w_qkv_eff[:, cs], w_qkv_eff[:, D:][:, cs],
             w_qkv_eff[:, 2 * D:][:, cs]], axis=1)
        bq = np.concatenate(
            [bias_qkv[cs], bias_qkv[D:][cs], bias_qkv[2 * D:][cs]])
        rows = shard_rows(c)
        in_maps.append({
            "xt": xt,
            "xsh": np.ascontiguousarray(xf[rows].astype(bf16)),
            "wqkv": np.ascontiguousarray(wq.astype(bf16)),
            "nws": np.ascontiguousarray(
                (-wq.astype(np.float32).sum(axis=0)).astype(
                    np.float32)).reshape(-1, 1),
            "bqkv": np.ascontiguousarray(
                np.asarray(bq, np.float32).reshape(3, 128).astype(bf16)),
            "wout": w_out_b,
            "w1": w1_eff,
            "b1g": b1g,
            "w2": w2_b,
            "masks": masks,
        })
    return in_maps


def kernel(**inputs):
    nc = build()
    in_maps = make_in_maps(**inputs)
    res = bass_utils.run_bass_kernel_spmd(
        nc, in_maps, core_ids=list(range(NCORES)))
    out = np.empty((T, D), np.float32)
    for c in range(NCORES):
        out[shard_rows(c)] = res.results[c]["out"]
    out += np.asarray(inputs["b2"], np.float32)[None, :]
    return out.reshape(B, L, D).astype(np.float32)


# revision 52
# speedup vs baseline: 1.4214x; 1.2018x over previous
"""Trainium2 8-core kernel for a dense pre-norm transformer block.

Reference: h=LN1(x); qkv=h@w_qkv; causal MHA (16 heads, Dh=64);
x+=o@w_out; h2=LN2(x); x+=gelu(h2@w1+b1)@w2+b2.

Sharding (v2 — collective-minimal):
  - heads 2c,2c+1 on core c (w_qkv column-shard); each core computes
    q/k/v for ALL tokens for its 2 heads and runs their attention.
  - residual stream token-sharded: core c owns the four strided pieces
    {q*1024 + c*128 .. +128}, q=0..3.
  - after attention, the normalized head-sharded output o is exchanged
    with ONE AllToAll per batch (each core ships [1024, 256] bf16 laid
    out as 8 per-destination shards and receives exactly its own 256
    tokens x all 1024 head-dims). Out-projection, LN2, and the ENTIRE
    MLP then run token-locally with full w_out/w1/w2 resident. No
    ReduceScatter / h2-AllGather anywhere (collectives: warmup AG +
    stats AG + 2 small A2As).
  - LN1 stats shard-local via bn_stats + tiny AllGather; LN1 folded
    into the qkv matmul (rank-1 mean correction + rstd col-scaling);
    ln gains folded into w_qkv/w1 host-side; b2 added host-side.

Compute dtype: bf16 operands, fp32 PSUM. Scores computed transposed
ST=[k_pos, q_pos]; both heads packed in the PE via tile_position and
share one [128,2,512] PSUM pair so softmax exp runs as ONE scalar
activation per k-chunk; softmax denominator via a ones-column appended
to V; causality via 4 static head-doubled [128,2,512] masks; V and h2
transposes run on the DMA XBAR, not the PE.
"""
import numpy as np

import concourse.bass as bass
import concourse.mybir as mybir
import concourse.tile as tile
from concourse import bacc
from concourse import bass_utils
from concourse.masks import make_identity

F32 = mybir.dt.float32
BF = mybir.dt.bfloat16
AF = mybir.ActivationFunctionType

NCORES = 8
B, L, D = 2, 2048, 1024
T = B * L              # 4096 tokens
TSH = T // NCORES      # 512 tokens per core (4 pieces of 128)
DH = 64                # head dim
HL = 2                 # heads per core
DLOC = HL * DH         # 128 local head features
LN_EPS = 1e-5
NT = T // 512          # 8 token tiles of 512
ND = D // 128          # 8 feature chunks
QT = L // 512          # 4 q-tiles per batch
NM = 4096 // 128       # 32 MLP hidden chunks

_CACHE = {}


def build():
    if "nc" in _CACHE:
        return _CACHE["nc"]
    nc = bacc.Bacc("TRN2", target_bir_lowering=False, debug=False,
                   num_devices=NCORES)

    xt_in = nc.dram_tensor("xt", [D, T], BF, kind="ExternalInput")
    xsh_in = nc.dram_tensor("xsh", [TSH, D], BF, kind="ExternalInput")
    wqkv_in = nc.dram_tensor("wqkv", [D, 3 * DLOC], BF, kind="ExternalInput")
    nws_in = nc.dram_tensor("nws", [3 * DLOC, 1], F32, kind="ExternalInput")
    bqkv_in = nc.dram_tensor("bqkv", [3, DLOC], BF, kind="ExternalInput")
    wout_in = nc.dram_tensor("wout", [D, D], BF, kind="ExternalInput")
    w1_in = nc.dram_tensor("w1", [D, 4096], BF, kind="ExternalInput")
    b1g_in = nc.dram_tensor("b1g", [128, NM], F32, kind="ExternalInput")
    w2_in = nc.dram_tensor("w2", [4096, D], BF, kind="ExternalInput")
    masks_in = nc.dram_tensor("masks", [4, 128, 512], BF, kind="ExternalInput")
    out_ext = nc.dram_tensor("out", [TSH, D], F32, kind="ExternalOutput")

    rg = [list(range(NCORES))]

    with tile.TileContext(nc) as tc:
        with (
            tc.tile_pool(name="const", bufs=1) as const,
            tc.tile_pool(name="wmain", bufs=1) as wmain,
            tc.tile_pool(name="dram", bufs=1, space="DRAM") as dram,
        ):
            # ---- DRAM scratch for collectives ----
            st_ag_in = dram.tile([8, 128], BF)       # (piece q, mean/rstd)
            st_ag_out = dram.tile([64, 128], BF, addr_space="Shared")
            oa2a_in = [dram.tile([D, 256], BF, name=f"oa2a_in{b}")
                       for b in range(B)]
            oa2a_out = [dram.tile([D, 256], BF, name=f"oa2a_out{b}")
                        for b in range(B)]

            # ---- constants ----
            ident_bf = const.tile([128, 128], BF)
            make_identity(nc, ident_bf[:])
            ones_row = const.tile([1, 128], BF)
            nc.vector.memset(ones_row[:], 1.0)
            ones512 = const.tile([1, 512], BF)
            nc.vector.memset(ones512[:], 1.0)
            eps128 = const.tile([128, 1], F32)
            nc.vector.memset(eps128[:], LN_EPS)
            # additive causal masks (0 / -800, pre-softmax-scale)
            masks_sb = [const.tile([128, 512], BF, name=f"mask{m}")
                        for m in range(4)]

            def load_masks():
                for m in range(4):
                    nc.gpsimd.dma_start(masks_sb[m][:], masks_in.ap()[m])

            # ---- persistent weights (full wout/w1; w2 in late pool) ----
            wout_sb = [wmain.tile([128, D], BF, name=f"wout{d}")
                       for d in range(ND)]
            w1_sb = [wmain.tile([128, 4096], BF, name=f"w1_{d}")
                     for d in range(ND)]
            b1g_sb = wmain.tile([128, NM], F32)

            def load_mid_weights():
                for d in range(ND):
                    nc.gpsimd.dma_start(wout_sb[d][:],
                                        wout_in.ap()[d * 128:(d + 1) * 128, :])
                for d in range(ND):
                    nc.gpsimd.dma_start(w1_sb[d][:],
                                        w1_in.ap()[d * 128:(d + 1) * 128, :])
                nc.gpsimd.dma_start(b1g_sb[:], b1g_in.ap())

            # persistent activations
            resid_pool_cm = tc.tile_pool(name="resid", bufs=1)
            resid_pool = resid_pool_cm.__enter__()
            xsv = resid_pool.tile([128, 4, D], BF)    # my shard of x
            x2_sb = resid_pool.tile([128, 4, D], BF)

            # ========== stage 0: shard-local LN1 stats + tiny AG ==========
            s0_cm = tc.tile_pool(name="s0", bufs=2)
            s0 = s0_cm.__enter__()
            ps0_cm = tc.tile_pool(name="ps0", bufs=2, space="PSUM")
            ps0 = ps0_cm.__enter__()
            for q in range(4):
                nc.scalar.dma_start(
                    xsv[:, q, :], xsh_in.ap()[q * 128:(q + 1) * 128, :])
                stats = s0.tile([128, 2, 6], F32, tag="stats", name="stats")
                xv = xsv[:, q, :].rearrange("p (s f) -> p s f", s=2)
                for s in range(2):
                    nc.vector.bn_stats(stats[:, s, :], xv[:, s, :])
                mv = s0.tile([128, 2], F32, tag="mv", name="mv")
                nc.vector.bn_aggr(mv[:], stats[:])
                rstd0 = s0.tile([128, 1], F32, tag="rstd0", name="rstd0")
                nc.scalar.activation(rstd0[:], mv[:, 1:2], AF.Sqrt,
                                     bias=eps128[:])
                nc.vector.reciprocal_approx_fast(rstd0[:], rstd0[:])
                st2 = s0.tile([128, 2], BF, tag="st2", name="st2")
                nc.vector.tensor_copy(st2[:, 0:1], mv[:, 0:1])
                nc.vector.tensor_copy(st2[:, 1:2], rstd0[:])
                stp = ps0.tile([2, 128], BF, tag="stp", name="stp")
                nc.tensor.transpose(stp[:], st2[:], ident_bf[:])
                sts = s0.tile([2, 128], BF, tag="sts", name="sts")
                nc.vector.tensor_copy(sts[:], stp[:])
                nc.scalar.dma_start(st_ag_in[2 * q:2 * q + 2, :], sts[:])
            nc.gpsimd.collective_compute(
                "AllGather", mybir.AluOpType.bypass, replica_groups=rg,
                ins=[st_ag_in[:].opt()], outs=[st_ag_out[:].opt()])
            ps0_cm.__exit__(None, None, None)
            s0_cm.__exit__(None, None, None)

            # st_ag_out rows: c*8 + q*2 + {0:mean, 1:rstd}
            st_view = st_ag_out[:].rearrange("(c x) f -> c x f", x=8)

            # ---- attention temp pools (entered early so the stage-1
            # pools above them on the stack can exit right after qkv) ----
            s2_vaug_cm = tc.tile_pool(name="s2_vaug", bufs=1)
            s2_vaug = s2_vaug_cm.__enter__()
            s2_exp_cm = tc.tile_pool(name="s2_exp", bufs=2)
            s2_exp = s2_exp_cm.__enter__()
            s2_misc_cm = tc.tile_pool(name="s2_misc", bufs=2)
            s2_misc = s2_misc_cm.__enter__()
            attn_pool_cm = tc.tile_pool(name="attn", bufs=1)
            attn_pool = attn_pool_cm.__enter__()
            qkvT = []
            for m in range(3):
                t_ = attn_pool.tile([128, T], BF, name=f"qkvT{m}")
                qkvT.append(t_)
            ps_bc_cm = tc.tile_pool(name="ps_bc", bufs=1, space="PSUM")
            ps_bc = ps_bc_cm.__enter__()

            # ================= stage 1: LN1 + qkv =================
            s1_w_cm = tc.tile_pool(name="s1_w", bufs=1)
            s1_w = s1_w_cm.__enter__()
            s1_x_cm = tc.tile_pool(name="s1_x", bufs=2)
            s1_x = s1_x_cm.__enter__()
            s1_tmp_cm = tc.tile_pool(name="s1_tmp", bufs=2)
            s1_tmp = s1_tmp_cm.__enter__()
            s1_stat_cm = tc.tile_pool(name="s1_stat", bufs=5)
            s1_stat = s1_stat_cm.__enter__()
            praw_pool_cm = tc.tile_pool(name="s1_praw", bufs=12)
            praw_pool = praw_pool_cm.__enter__()
            ps_qkv_cm = tc.tile_pool(name="ps_qkv", bufs=2, space="PSUM")
            ps_qkv = ps_qkv_cm.__enter__()
            praws = {}

            # qkv weight shard in SBUF (stage-1 scope only)
            wqkv_sb = []
            for d in range(ND):
                wt = s1_w.tile([128, 3 * DLOC], BF, name=f"wqkv{d}")
                nc.gpsimd.dma_start(wt[:],
                                    wqkv_in.ap()[d * 128:(d + 1) * 128, :])
                wqkv_sb.append(wt)
            nws_sb = []
            for m in range(3):
                nt_ = s1_w.tile([128, 1], F32, name=f"nws{m}")
                nc.gpsimd.dma_start(nt_[:],
                                    nws_in.ap()[m * 128:(m + 1) * 128, :])
                nws_sb.append(nt_)
            bqkv_sb = []
            for m in range(3):
                bt = s1_w.tile([1, DLOC], BF, name=f"bqkv{m}")
                nc.gpsimd.dma_start(bt[:], bqkv_in.ap()[m:m + 1, :])
                bqkv_sb.append(bt)

            def do_s1_mm(tt):
                q4, h4 = tt // 2, tt % 2
                xts = s1_x.tile([128, ND, 512], BF, tag="xts")
                for d in range(ND):
                    eng = nc.sync if d % 2 == 0 else nc.scalar
                    eng.dma_start(
                        xts[:, d, :],
                        xt_in.ap()[d * 128:(d + 1) * 128,
                                   tt * 512:(tt + 1) * 512])
                st_bf = s1_stat.tile([1, 2, 4, 128], BF, tag="st_bf",
                                     name=f"st_bf{tt}")
                # on gpsimd: keeps the stats-AG wait OFF the xt-load queues
                nc.gpsimd.dma_start(
                    st_bf[:, 0, :, :], st_view[4 * h4:4 * h4 + 4, 2 * q4, :])
                nc.gpsimd.dma_start(
                    st_bf[:, 1, :, :],
                    st_view[4 * h4:4 * h4 + 4, 2 * q4 + 1, :])
                praws[tt] = (st_bf, [])
                for m in range(3):
                    ps_q = ps_qkv.tile([128, 512], F32, tag="ps_q",
                                       name="ps_q")
                    for d in range(ND):
                        nc.tensor.matmul(
                            ps_q[:], wqkv_sb[d][:, m * 128:(m + 1) * 128],
                            xts[:, d, :], start=(d == 0), stop=False)
                    # rank-1 bias add: bqkv[m] x ones
                    nc.tensor.matmul(ps_q[:], bqkv_sb[m][:], ones512[:],
                                     start=False, stop=True)
                    praw = praw_pool.tile([128, 512], BF, tag="praw",
                                          name="praw")
                    nc.scalar.copy(praw[:], ps_q[:])
                    praws[tt][1].append(praw)

            def do_s1_fin(tt):
                st_bf, praw3 = praws[tt]
                mean_v = st_bf[:, 0, :, :].rearrange("p a f -> p (a f)")
                rstd_v = st_bf[:, 1, :, :].rearrange("p a f -> p (a f)")
                mr = s1_stat.tile([1, 512], BF, tag="mr", name="mr", bufs=1)
                nc.vector.tensor_mul(mr[:], mean_v, rstd_v)
                rstd_b = ps_bc.tile([128, 512], F32, tag="pbc",
                                    name="rstd_b")
                nc.tensor.matmul(rstd_b[:], ones_row[:], rstd_v,
                                 start=True, stop=True)
                rstd_bc = s1_tmp.tile([128, 512], BF, tag="rstd_bc")
                nc.vector.tensor_copy(rstd_bc[:], rstd_b[:])
                mr_b = ps_bc.tile([128, 512], F32, tag="pbc", name="mr_b")
                nc.tensor.matmul(mr_b[:], ones_row[:], mr[:],
                                 start=True, stop=True)
                for m in range(3):
                    u = s1_tmp.tile([128, 512], BF, tag="pre", name="u")
                    nc.vector.tensor_mul(u[:], praw3[m][:], rstd_bc[:])
                    nc.vector.scalar_tensor_tensor(
                        out=qkvT[m][:, tt * 512:(tt + 1) * 512],
                        in0=mr_b[:], scalar=nws_sb[m][:],
                        in1=u[:], op0=mybir.AluOpType.mult,
                        op1=mybir.AluOpType.add)
                del praws[tt]

            # ---------------- stage-1 front schedule ----------------
            load_masks()
            load_mid_weights()
            for tt in range(4):
                do_s1_mm(tt)
            do_s1_fin(0)
            do_s1_mm(4)
            do_s1_fin(1)
            do_s1_mm(5)
            do_s1_fin(2)
            do_s1_mm(6)
            do_s1_fin(3)
            do_s1_mm(7)
            ps_qkv_cm.__exit__(None, None, None)

            # ============ attention ============
            ps_pair_cm = tc.tile_pool(name="ps_pair", bufs=2, space="PSUM")
            ps_pair = ps_pair_cm.__enter__()
            ps_o_cm = tc.tile_pool(name="ps_o", bufs=1, space="PSUM")
            ps_o = ps_o_cm.__enter__()
            ps_vtr_cm = tc.tile_pool(name="ps_vtr", bufs=1, space="PSUM")
            ps_vtr = ps_vtr_cm.__enter__()

            vaug_cur = [None]

            def do_attn(b, js):
                tok0 = b * L
                if js[0] == 0:
                    vaug = s2_vaug.tile([128, 2, L // 128, DH + 1], BF,
                                        tag="vaug", name="vaug")
                    nc.vector.memset(vaug[:, :, :, DH:DH + 1], 1.0)
                    vaug_cur[0] = vaug
                vaug = vaug_cur[0]
                for j in js:
                    # V transpose on the PE, both heads at once, lazily:
                    # j needs k-chunks < 4(j+1); chunks 4j..4j+3 are new
                    for kc in range(4 * j, 4 * (j + 1)):
                        vtr = ps_vtr.tile([128, 128], BF, tag="vtr",
                                          name="vtr")
                        nc.tensor.transpose(
                            vtr[:],
                            qkvT[2][:, tok0 + kc * 128:tok0 + (kc + 1) * 128],
                            ident_bf[:])
                        nc.vector.tensor_copy(
                            vaug[:, :, kc, 0:DH],
                            vtr[:].rearrange("p (h d) -> p h d", h=2))
                    nk = 4 * (j + 1)
                    po = [ps_o.tile([DH + 1, 512], F32, tag=f"po{hl}",
                                    name=f"po{hl}") for hl in range(HL)]
                    for kc in range(nk):
                        dm = kc - (nk - 4)
                        col0 = 128 * dm if dm > 0 else 0
                        w = 512 - col0
                        pst2 = ps_pair.tile([128, 2, 512], F32, tag="pst2",
                                            name="pst2")
                        for hl in range(HL):
                            hrow = hl * DH
                            qsl = qkvT[0][hrow:hrow + DH,
                                          tok0 + j * 512 + col0:
                                          tok0 + (j + 1) * 512]
                            ksl = qkvT[1][hrow:hrow + DH,
                                          tok0 + kc * 128:
                                          tok0 + (kc + 1) * 128]
                            nc.tensor.matmul(pst2[:, hl, :w], ksl, qsl,
                                             start=True, stop=(dm < 0),
                                             tile_position=(hrow, 0))
                            if dm >= 0:
                                # additive -800 causal mask via the PE
                                nc.tensor.matmul(
                                    pst2[:, hl, :w], ident_bf[:],
                                    masks_sb[dm][:, col0:],
                                    start=False, stop=True)
                        est2 = s2_exp.tile([128, 2, 512], BF, tag="est2",
                                           name="est2")
                        nc.scalar.activation(est2[:, :, :w], pst2[:, :, :w],
                                             AF.Exp, scale=0.125)
                        for hl in range(HL):
                            nc.tensor.matmul(po[hl][:, col0:],
                                             vaug[:, hl, kc, :],
                                             est2[:, hl, :w],
                                             start=(kc == 0),
                                             stop=(kc == nk - 1))
                    par = j // 2
                    for hl in range(HL):
                        # copy po out of PSUM immediately: frees the bank
                        # for the next j-tile and speeds downstream reads
                        po_sb = s2_misc.tile([DH + 1, 512], BF,
                                             tag=f"posb{hl}", name="po_sb",
                                             bufs=1)
                        nc.vector.tensor_copy(po_sb[:], po[hl][:])
                        # NOTE: reciprocal_approx_fast (custom DVE op)
                        # cannot read PSUM — and needs fp32 input.
                        den = s2_misc.tile([1, 512], F32, tag="den",
                                           name="den", bufs=1)
                        nc.vector.tensor_copy(den[:], po_sb[DH:DH + 1, :])
                        rec1 = s2_misc.tile([1, 512], F32, tag="rec1",
                                            name="rec1", bufs=1)
                        nc.vector.reciprocal_approx_fast(rec1[:], den[:])
                        rec1b = s2_misc.tile([1, 512], BF, tag="rec1b",
                                             name="rec1b", bufs=1)
                        nc.scalar.copy(rec1b[:], rec1[:])
                        rec_sb = s2_misc.tile([64, 512], BF, tag="rec_sb",
                                              name="rec_sb")
                        nc.gpsimd.partition_broadcast(rec_sb[:], rec1b[:])
                        osl = s2_misc.tile([64, 512], BF, tag="osl",
                                           name="osl")
                        nc.vector.tensor_mul(osl[:], po_sb[0:DH, :],
                                             rec_sb[:])
                        # scatter the 4 destination-core shards:
                        # dst rows (jmod4+s)*128 + hl*64 .. +64,
                        # dst cols par*128 .. +128
                        s4base = (j % 2) * 4
                        dst = oa2a_in[b][:].rearrange(
                            "(s r) t -> r s t", r=128)[
                                hl * DH:(hl + 1) * DH,
                                s4base:s4base + 4,
                                par * 128:(par + 1) * 128]
                        src = osl[:].rearrange("d (s t) -> d s t", s=4)
                        nc.gpsimd.dma_start(dst, src)

            def fire_oa2a(b):
                nc.gpsimd.collective_compute(
                    "AllToAll", mybir.AluOpType.bypass, replica_groups=rg,
                    ins=[oa2a_in[b][:].opt()], outs=[oa2a_out[b][:].opt()])

            # ---------------- attention schedule ----------------
            do_attn(0, (0, 1))
            for tt in range(4, NT):
                do_s1_fin(tt)
            do_attn(0, (2, 3))
            fire_oa2a(0)
            do_attn(1, (0, 1, 2, 3))
            fire_oa2a(1)

            for cm in (ps_vtr_cm, ps_o_cm, ps_pair_cm, praw_pool_cm,
                       s1_stat_cm, s1_tmp_cm, s1_x_cm, s1_w_cm, ps_bc_cm,
                       attn_pool_cm):
                cm.__exit__(None, None, None)

            # ---- stages 3+: out-proj + LN2 + token-local MLP ----
            w2p_cm = tc.tile_pool(name="w2p", bufs=1)
            w2p = w2p_cm.__enter__()
            w2_sb = [w2p.tile([128, D], BF, name=f"w2_{m}")
                     for m in range(NM)]
            for m in range(NM):
                eng = nc.sync if m % 2 == 0 else nc.gpsimd
                eng.dma_start(w2_sb[m][:], w2_in.ap()[m * 128:(m + 1) * 128, :])

            s3_cm = tc.tile_pool(name="s3", bufs=2)
            s3 = s3_cm.__enter__()
            s3h_cm = tc.tile_pool(name="s3h", bufs=2)
            s3h = s3h_cm.__enter__()
            g1_cm = tc.tile_pool(name="g1", bufs=4)
            g1p = g1_cm.__enter__()
            out_cm = tc.tile_pool(name="outp", bufs=2)
            outp = out_cm.__enter__()
            ps_pop_cm = tc.tile_pool(name="ps_pop", bufs=1, space="PSUM")
            ps_pop = ps_pop_cm.__enter__()
            ps_m1_cm = tc.tile_pool(name="ps_m1", bufs=2, space="PSUM")
            ps_m1 = ps_m1_cm.__enter__()
            ps_m2_cm = tc.tile_pool(name="ps_m2", bufs=1, space="PSUM")
            ps_m2 = ps_m2_cm.__enter__()

            h2Ts = {}

            def og_load(q, eng):
                b, par = q // 2, q % 2
                og = s3.tile([128, ND, 128], BF, tag="og", name="og")
                for dc in range(ND):
                    eng.dma_start(
                        og[:, dc, :],
                        oa2a_out[b][dc * 128:(dc + 1) * 128,
                                    par * 128:(par + 1) * 128])
                return og

            def do_oproj(q, og):
                """out-proj + residual + LN2 + h2T for my piece q."""
                for n in range(2):
                    pop = ps_pop.tile([128, 512], F32, tag=f"pop{n}",
                                      name=f"pop{n}")
                    for dc in range(ND):
                        nc.tensor.matmul(
                            pop[:], og[:, dc, :],
                            wout_sb[dc][:, n * 512:(n + 1) * 512],
                            start=(dc == 0), stop=(dc == ND - 1))
                    nc.vector.tensor_add(
                        x2_sb[:, q, n * 512:(n + 1) * 512],
                        xsv[:, q, n * 512:(n + 1) * 512], pop[:])
                stats = s3.tile([128, 2, 6], F32, tag="stats", name="stats")
                x2v = x2_sb[:, q, :].rearrange("p (s f) -> p s f", s=2)
                for s in range(2):
                    nc.vector.bn_stats(stats[:, s, :], x2v[:, s, :])
                mv = s3.tile([128, 2], F32, tag="mv", name="mv")
                nc.vector.bn_aggr(mv[:], stats[:])
                rstd2 = s3.tile([128, 1], F32, tag="rstd2", name="rstd2")
                nc.scalar.activation(rstd2[:], mv[:, 1:2], AF.Sqrt,
                                     bias=eps128[:])
                nc.vector.reciprocal_approx_fast(rstd2[:], rstd2[:])
                h2b = s3.tile([128, D], BF, tag="h2b", name="h2b")
                nc.vector.tensor_scalar(
                    out=h2b[:], in0=x2_sb[:, q, :], scalar1=mv[:, 0:1],
                    scalar2=rstd2[:], op0=mybir.AluOpType.subtract,
                    op1=mybir.AluOpType.mult)
                h = q // 2
                if h not in h2Ts:
                    h2Ts[h] = s3h.tile([128, ND, 256], BF, tag="h2T",
                                       name="h2T")
                h2T = h2Ts[h]
                poff = (q % 2) * 128
                for dc in range(ND):
                    eng = nc.sync if dc % 2 == 0 else nc.scalar
                    eng.dma_start_transpose(
                        h2T[:, dc, poff:poff + 128],
                        h2b[:, dc * 128:(dc + 1) * 128])

            def do_mlp_half(h):
                """MLP for pieces 2h, 2h+1 (my 256 tokens of batch h)."""
                h2T = h2Ts[h]
                pm2 = {}
                for p in range(2):
                    for n2 in range(2):
                        pm2[(p, n2)] = ps_m2.tile(
                            [128, 512], F32, tag=f"pm2_{p}{n2}",
                            name=f"pm2_{p}{n2}")
                g1_prev = [None]

                def mlp2(m, g1):
                    for p in range(2):
                        for n2 in range(2):
                            nc.tensor.matmul(
                                pm2[(p, n2)][:], g1[:, p * 128:(p + 1) * 128],
                                w2_sb[m][:, n2 * 512:(n2 + 1) * 512],
                                start=(m == 0), stop=(m == NM - 1))

                for m in range(NM):
                    pm1 = ps_m1.tile([128, 256], F32, tag="pm1", name="pm1")
                    for d in range(ND):
                        nc.tensor.matmul(
                            pm1[:], w1_sb[d][:, m * 128:(m + 1) * 128],
                            h2T[:, d, :], start=(d == 0), stop=(d == ND - 1))
                    g1 = g1p.tile([128, 256], BF, tag="g1", name="g1")
                    nc.scalar.activation(g1[:], pm1[:], AF.Gelu,
                                         bias=b1g_sb[:, m:m + 1])
                    if m > 0:
                        mlp2(m - 1, g1_prev[0])
                    g1_prev[0] = g1
                mlp2(NM - 1, g1_prev[0])
                for p in range(2):
                    q = 2 * h + p
                    for n2 in range(2):
                        ot = outp.tile([128, 512], F32, tag="ot", name="ot")
                        nc.vector.tensor_add(
                            ot[:], x2_sb[:, q, n2 * 512:(n2 + 1) * 512],
                            pm2[(p, n2)][:])
                        nc.sync.dma_start(
                            out_ext.ap()[q * 128:(q + 1) * 128,
                                         n2 * 512:(n2 + 1) * 512], ot[:])

            og0 = og_load(0, nc.scalar)
            og1 = og_load(1, nc.scalar)
            do_oproj(0, og0)
            do_oproj(1, og1)
            og2 = og_load(2, nc.gpsimd)
            og3 = og_load(3, nc.gpsimd)
            do_mlp_half(0)
            do_oproj(2, og2)
            do_oproj(3, og3)
            do_mlp_half(1)

            for cm in (ps_m2_cm, ps_m1_cm, ps_pop_cm, out_cm, g1_cm,
                       s3h_cm, s3_cm, w2p_cm, s2_misc_cm, s2_exp_cm,
                       s2_vaug_cm, resid_pool_cm):
                cm.__exit__(None, None, None)

    nc.compile()
    _CACHE["nc"] = nc
    return nc


def shard_rows(c):
    """Global token rows owned by core c (four strided pieces of 128)."""
    return np.concatenate(
        [np.arange(q * 1024 + c * 128, q * 1024 + (c + 1) * 128)
         for q in range(4)])


def make_in_maps(x, ln1_g, ln1_b, w_qkv, w_out, ln2_g, ln2_b, w1, b1, w2, b2):
    import ml_dtypes
    bf16 = ml_dtypes.bfloat16
    x = np.asarray(x, np.float32)
    xf = np.ascontiguousarray(x.reshape(T, D))
    xt = np.ascontiguousarray(xf.T.astype(bf16))
    w_qkv_eff = np.asarray(w_qkv) * np.asarray(ln1_g)[:, None]
    bias_qkv = np.asarray(ln1_b) @ np.asarray(w_qkv)
    w1_eff = np.ascontiguousarray(
        (np.asarray(w1) * np.asarray(ln2_g)[:, None]).astype(bf16))
    bias_h1 = np.asarray(ln2_b) @ np.asarray(w1) + np.asarray(b1)
    b1g = np.ascontiguousarray(
        bias_h1.reshape(NM, 128).T.astype(np.float32))
    w_out_b = np.ascontiguousarray(np.asarray(w_out).astype(bf16))
    w2_b = np.ascontiguousarray(np.asarray(w2).astype(bf16))
    km = np.arange(128)[:, None]
    qm = np.arange(512)[None, :]
    # additive masks: 0 where attended, -800 where causally masked
    masks = np.stack([
        np.where(km + 128 * m <= qm, 0.0, -800.0).astype(bf16)
        for m in range(4)])
    in_maps = []
    for c in range(NCORES):
        cs = slice(c * DLOC, (c + 1) * DLOC)
        wq = np.concatenate(
            [w_qkv_eff[:, cs], w_qkv_eff[:, D:][:, cs],
             w_qkv_eff[:, 2 * D:][:, cs]], axis=1)
        bq = np.concatenate(
            [bias_qkv[cs], bias_qkv[D:][cs], bias_qkv[2 * D:][cs]])
        rows = shard_rows(c)
        in_maps.append({
            "xt": xt,
            "xsh": np.ascontiguousarray(xf[rows].astype(bf16)),
            "wqkv": np.ascontiguousarray(wq.astype(bf16)),
            "nws": np.ascontiguousarray(
                (-wq.astype(np.float32).sum(axis=0)).astype(
                    np.float32)).reshape(-1, 1),
            "bqkv": np.ascontiguousarray(
                np.asarray(bq, np.float32).reshape(3, 128).astype(bf16)),
            "wout": w_out_b,
            "w1": w1_eff,
            "b1g": b1g,
            "w2": w2_b,
            "masks": masks,
        })
    return in_maps


def kernel(**inputs):
    nc = build()
    in_maps = make_in_maps(**inputs)
    res = bass_utils.run_bass_kernel_spmd(
        nc, in_maps, core_ids=list(range(NCORES)))
    out = np.empty((T, D), np.float32)
    for c in range(NCORES):
        out[shard_rows(c)] = res.results[c]["out"]
    out += np.asarray(inputs["b2"], np.float32)[None, :]
    return out.reshape(B, L, D).astype(np.float32)
